# revision 10
# baseline (speedup 1.0000x reference)
"""Segment-sum (AggrSum) kernel for 8 Trainium2 NeuronCores.

Math: out[v, :] = sum_{n: X_neis[n] == v} H[n, :]   (H [N, D], out [V, D])

Strategy (V-sharding with host-side bucketing as the sharding step):
  - Sort edge ids by their target vocab index (stable argsort on host).
  - Core c owns vocab rows [c*V/8, (c+1)*V/8) -> 8 vocab tiles of 128 rows.
  - For each 128-row vocab tile, its edges are padded to K tiles of 128
    edges (K = max over vocab tiles, compile-time constant).
  - Per core the device kernel streams the pre-bucketed H rows, builds a
    one-hot mask M[e, v] = (xrel[e] == v) with a DVE is_equal against a
    static iota row, and matmul-accumulates M.T @ H_tile into PSUM.
    Padded slots carry xrel = -1 (matches nothing) and zero H rows.
  - Each core writes its own [V/8, D] output slice; no cross-core
    reduction is needed. Host concatenates the 8 slices.
"""

import numpy as np

import concourse.bacc as bacc
import concourse.bass as bass
import concourse.mybir as mybir
import concourse.tile as tile
from concourse.bass_utils import run_bass_kernel_spmd

N, D, V = 32768, 256, 8192
NCORES = 8
P = 128
VT_PER_CORE = V // P // NCORES  # 8 vocab tiles of 128 per core

TRACE = False
LAST_EXEC_NS = None
LAST_RESULTS = None

_PROGRAM_CACHE: dict = {}


def _build_program(K: int):
    """Bass program for one core: 8 vocab tiles, K edge tiles each."""
    f32 = mybir.dt.float32
    nconst = VT_PER_CORE * K + P
    nc = bacc.Bacc("TRN2", target_bir_lowering=False)
    hs = nc.dram_tensor("hs", [P, VT_PER_CORE * K * D], f32, kind="ExternalInput")
    # consts packs [xrel | iota] so the first is_equal waits on one DMA sem
    consts = nc.dram_tensor("consts", [P, nconst], f32, kind="ExternalInput")
    out = nc.dram_tensor("out", [VT_PER_CORE * P, D], f32, kind="ExternalOutput")

    with tile.TileContext(nc) as tc:
        with (
            tc.tile_pool(name="const", bufs=1) as const_pool,
            tc.tile_pool(name="h", bufs=3) as hpool,
            tc.tile_pool(name="m", bufs=4) as mpool,
            tc.tile_pool(name="o", bufs=2) as opool,
            tc.tile_pool(name="psum", bufs=4, space="PSUM") as psum_pool,
        ):
            const_sb = const_pool.tile([P, nconst], f32)
            nc.sync.dma_start(const_sb[:], consts[:])
            iota_off = VT_PER_CORE * K

            for vt in range(VT_PER_CORE):
                ht = hpool.tile([P, K * D], f32)
                nc.sync.dma_start(ht[:], hs[:, vt * K * D : (vt + 1) * K * D])
                ps = psum_pool.tile([P, D], f32)
                for k in range(K):
                    t = vt * K + k
                    m = mpool.tile([P, P], f32)
                    nc.vector.tensor_tensor(
                        out=m[:],
                        in0=const_sb[:, t : t + 1].to_broadcast([P, P]),
                        in1=const_sb[:, iota_off : iota_off + P],
                        op=mybir.AluOpType.is_equal,
                    )
                    nc.tensor.matmul(
                        out=ps[:],
                        lhsT=m[:],
                        rhs=ht[:, k * D : (k + 1) * D],
                        start=(k == 0),
                        stop=(k == K - 1),
                    )
                ot = opool.tile([P, D], f32)
                nc.scalar.copy(ot[:], ps[:])
                nc.sync.dma_start(out[vt * P : (vt + 1) * P, :], ot[:])
    nc.finalize()
    return nc


def _shard_inputs(H: np.ndarray, X: np.ndarray):
    """Bucket edges by vocab tile; build per-core padded input maps."""
    n_vt_global = V // P  # 64
    order = np.argsort(X, kind="stable")
    Xs = X[order]
    counts = np.bincount(X, minlength=V).reshape(n_vt_global, P).sum(axis=1)
    starts = np.zeros(n_vt_global + 1, dtype=np.int64)
    np.cumsum(counts, out=starts[1:])
    K = max(1, int(-(-counts.max() // P)))

    iota_np = np.tile(np.arange(P, dtype=np.float32), (P, 1))

    in_maps = []
    for c in range(NCORES):
        hs = np.zeros((P, VT_PER_CORE * K * D), dtype=np.float32)
        xr = np.full((P, VT_PER_CORE * K), -1.0, dtype=np.float32)
        for vt in range(VT_PER_CORE):
            g = c * VT_PER_CORE + vt
            s, e = int(starts[g]), int(starts[g + 1])
            cnt = e - s
            rows = order[s:e]
            # block [cnt, D] -> partition p holds rows k*P+p as column
            # blocks k of width D
            block = np.zeros((K * P, D), dtype=np.float32)
            block[:cnt] = H[rows]
            blk = block.reshape(K, P, D).transpose(1, 0, 2).reshape(P, K * D)
            hs[:, vt * K * D : (vt + 1) * K * D] = blk
            xv = np.full(K * P, -1.0, dtype=np.float32)
            xv[:cnt] = (Xs[s:e] - g * P).astype(np.float32)
            xr[:, vt * K : (vt + 1) * K] = xv.reshape(K, P).T
        in_maps.append({"hs": hs, "consts": np.hstack([xr, iota_np])})
    return K, in_maps


def kernel(H, X_neis, V=V):
    global LAST_EXEC_NS, LAST_RESULTS
    H = np.asarray(H, dtype=np.float32)
    X = np.asarray(X_neis).astype(np.int64)
    assert H.shape == (N, D) and X.shape == (N,)

    K, in_maps = _shard_inputs(H, X)
    if K not in _PROGRAM_CACHE:
        _PROGRAM_CACHE[K] = _build_program(K)
    nc = _PROGRAM_CACHE[K]

    res = run_bass_kernel_spmd(nc, in_maps, list(range(NCORES)), trace=TRACE)
    LAST_EXEC_NS = res.exec_time_ns
    LAST_RESULTS = res
    return np.concatenate([res.results[c]["out"] for c in range(NCORES)], axis=0)
